# revision 19
# baseline (speedup 1.0000x reference)
"""KoLeo loss kernel for 8 Trainium2 NeuronCores — symmetric-half version.

Reference computation (B=16384, D=1024):
    xn  = x / max(||x||_2, 1e-12)          # row L2-normalize
    sim = xn @ xn.T                        # B x B cosine similarity
    max_sim[i] = max_{j != i} sim[i, j]    # nearest neighbor (excl. self)
    out = -mean(log(sqrt(2 - 2*max_sim + 1e-8)))

Sharding + symmetry: rows of x are split across 8 cores (2048 rows each).
sim is symmetric, so each computed entry sim[i, j] can serve both row i's
max (row-max over the streamed block) and row j's max (column-max,
accumulated across row chunks).  Each 128-row chunk m of a core computes
only an 8320-column window starting at its own diagonal (rotated frame),
instead of the full 16384 columns.  8320 is the provable minimum for a
diagonal-anchored window: a pair {a, b} is missed by chunk(a) iff
(a%128 + delta) mod B in [C, B) and by chunk(b) iff delta in
(b%128, b%128 + B - C]; both fail only if 2C <= B + a%128 + b%128
<= 16638, so C = 8320 (2C = 16640) covers every pair.  This cuts the
matmul work to 0.508x of the full matrix (the true lower bound is 0.5).

Engine pipeline per psum block (width 512; a 128-wide tail per chunk):
    PE   : 4 fp8-DoubleRow matmuls  -> ps [128, w] f32 (PSUM) (853 ns)
    Act  : copy ps -> blk [128, w] fp16 (SBUF)                (~675 ns)
    DVE  : (t==0: blk[:, :128] += -2*scale^2*I)
           rowbuf_m = max(rowbuf_m, blk)   (row-candidate accumulate)
           acc[:, s:s+w] = max(blk, acc)   (column-max)
Both DVE ops are elementwise tensor_max (TensorTensor 2x tier on fp16
SBUF, ~325 ns) — reduce-class DVE ops are capped at 1x (~575 ns), so
the row direction also accumulates elementwise into a per-chunk
[128, 512] buffer and is reduced ONCE per chunk at the end.
The fp16 bounce copy gives the DVE 2-byte SBUF operands (2x/4x tiers;
GPSIMD cannot run TensorTensor ops and cannot read PSUM on TRN2).
Row-max reduces only the fp16 block (never the cross-chunk
accumulator), so it is exact: the accumulator holds other rows' maxima
and must not leak into row-max.

Host finishes: per-column partition max of acc, scatter-max into the
global per-row max (the rotation makes that two slice maxes per core),
then the scalar log/sqrt/mean epilogue in float64.

Implementation notes:
  - Host pre-normalizes rows (f64) and pre-transposes to x.T in fp8e4m3
    scaled by 8 (DoubleRow perf mode, 2x matmul throughput; similarities
    come out scaled by 64, undone on the host).
  - Per-core input is x.T rotated so the core's own rows are columns
    0..2047; chunk m's window starts at column 128*m, so the
    self-similarity diagonal sits at window offset [0, 128) for every
    chunk — one -2*scale^2*I add per chunk kills the self-match.
  - Blocks are processed sorted by window-start column so the resident
    rhs DMA streams strictly left-to-right and finalized slices of the
    column-max accumulator DMA out while compute continues (no tail).
  - One resident SBUF tensor R [128, 8, 10240] (fp8) serves as both
    matmul weights (own rows = cols 0..2047) and moving data.  The
    input DMA streams in ~1-2K-column pieces: wide per-partition
    descriptor runs keep the DMA rings efficient (512-B descriptor
    runs measured 2.4x slower aggregate and stalled the PE ~58 us).
"""

import sys

if "/opt/trn_rl_repo" not in sys.path:
    sys.path.insert(0, "/opt/trn_rl_repo")

import os as _os

import numpy as np
import ml_dtypes

import concourse.bass as bass  # noqa: F401  (import keeps bass registered)
import concourse.mybir as mybir
import concourse.tile as tile
from concourse import bacc
from concourse.bass_utils import run_bass_kernel_spmd

P = 128          # SBUF partitions
NBLK = 512       # similarity column block width (= one PSUM bank of f32)
EPS = 1e-8

B = 16384        # rows of x
D = 1024         # feature dim
N_CORES = 8
BL = B // N_CORES          # local rows per core (2048)
MCH = BL // P              # row chunks per core (16)
WCOLS = 16 * NBLK + P      # window columns per chunk (8320, the minimum)
NT = 17                    # blocks per chunk window (16 full + 128 tail)
CW = (MCH - 1) * P + WCOLS  # resident rotated columns (10240)
KCH = D // P               # contraction chunks (8)
KSTEP = 2                  # fp8 DoubleRow: K chunks of 256

FP8_SCALE = 8.0
NEG_INIT = -60000.0        # fp16-representable, far below any -128..128 sim

# row-reduce flavor: "ts" = tensor_scalar w/ max-accum (fast path),
# "reduce" = plain reduce_max (1x, known-good fallback)
ROWRED = _os.environ.get("KOLEO_ROWRED", "ts")
MEMSET_ENG = _os.environ.get("KOLEO_MEMSET", "vector")  # "gpsimd" | "vector"

# input DMA piece boundaries (columns): fine first, then wide
_PIECES = [0, 1024, 3072, 5120, 7168, CW]


def build_nc(n_cores=N_CORES):
    """Build the per-core SPMD Bass program.

    Inputs :  xt     [D, CW] fp8e4m3 — rotated, normalized, scaled x.T
              negeye [P, P]  f16 — the constant -2*scale^2 * I
    Outputs:  rowmax [P, MCH] f32 — rowmax[p, m] = scale^2 *
              max_{j in window} sim[128m+p, j] (excl. self)
              colacc [P, CW] f16 — colacc[p, c] = scale^2 *
              max over chunks m (with c in window m) of sim[128m+p, c]
    """
    in_dt = mybir.dt.float8e4
    f32 = mybir.dt.float32
    f16 = mybir.dt.float16
    perf_mode = mybir.MatmulPerfMode.DoubleRow

    nc = bacc.Bacc("TRN2", target_bir_lowering=False, debug=False,
                   num_devices=n_cores)
    xt = nc.dram_tensor("xt", [D, CW], in_dt, kind="ExternalInput")
    negeye = nc.dram_tensor("negeye", [P, P], f16, kind="ExternalInput")
    rowmax_out = nc.dram_tensor("rowmax", [P, MCH], f32,
                                kind="ExternalOutput")
    colacc_out = nc.dram_tensor("colacc", [P, CW], f16,
                                kind="ExternalOutput")
    xt_ap = xt[:]

    with tile.TileContext(nc) as tc:
        with (
            tc.tile_pool(name="data", bufs=1) as data_pool,
            tc.tile_pool(name="blk", bufs=8) as blk_pool,
            tc.tile_pool(name="psum", bufs=8, space="PSUM") as psum_pool,
            tc.tile_pool(name="stats", bufs=1) as stats_pool,
        ):
            R = data_pool.tile([P, KCH, CW], in_dt, name="R")
            acc = data_pool.tile([P, CW], f16, name="acc")
            eye = stats_pool.tile([P, P], f16, name="eye")
            rowbufs = [
                stats_pool.tile([P, NBLK], f16, name=f"rowbuf{m}",
                                tag=f"rowbuf{m}")
                for m in range(MCH)
            ]
            rowmax = stats_pool.tile([P, MCH], f32, name="rowmax")

            # col-max accumulator starts far below any similarity
            memset_eng = nc.gpsimd if MEMSET_ENG == "gpsimd" else nc.vector
            memset_eng.memset(acc[:], NEG_INIT)
            nc.sync.dma_start(eye[:], negeye[:])

            # Stream the rotated slab left-to-right on the two HWDGE
            # queues (sync + scalar); gpsimd software DGE stays free.
            dma_eng = [nc.sync, nc.scalar]
            di = 0
            for j in range(len(_PIECES) - 1):
                c0, c1 = _PIECES[j], _PIECES[j + 1]
                for k in range(KCH):
                    dma_eng[di % 2].dma_start(
                        R[:, k, c0:c1], xt_ap[k * P:(k + 1) * P, c0:c1])
                    di += 1

            # Work items sorted by window-start column: each full item is
            # a PAIR of adjacent 512-col blocks of the same chunk (the DVE
            # then folds the pair once, accumulates rowbuf once, and runs
            # ONE 1024-wide column-max — 3 ops/1024 cols instead of 4,
            # cutting per-op overhead), plus one 128-col tail per chunk.
            # The tail's column-max is provably redundant: its pairs
            # (window cols >= 8192 = 8064 + 128 > 8064 + i') are always
            # computed by the partner chunk too, whose row-max covers the
            # column's row (verified exact in numpy).
            items = sorted(
                [(P * m + 1024 * q, m, q, False)
                 for m in range(MCH) for q in range(8)] +
                [(P * m + 8192, m, 8, True) for m in range(MCH)]
            )
            tmp = stats_pool.tile([P, NBLK], f16, name="tmp")
            dma_ptr = 0
            flush_i = 0
            for idx, (start, m, q, is_tail) in enumerate(items):
                lhsT0 = R[:, 0:KSTEP, m * P:(m + 1) * P]
                if is_tail:
                    w = P
                    ps = psum_pool.tile([P, NBLK], f32, name="ps", tag="ps")
                    for g in range(KCH // KSTEP):
                        k = g * KSTEP
                        nc.tensor.matmul(
                            ps[:, 0:w],
                            R[:, k:k + KSTEP, m * P:(m + 1) * P],
                            R[:, k:k + KSTEP, start:start + w],
                            start=(g == 0), stop=(k + KSTEP == KCH),
                            perf_mode=perf_mode,
                        )
                    blk = blk_pool.tile([P, 2 * NBLK], f16, name="blk",
                                        tag="blk")
                    nc.scalar.copy(blk[:, 0:w], ps[:, 0:w])
                    nc.vector.tensor_max(
                        out=rowbufs[m][:, 0:w],
                        in0=blk[:, 0:w],
                        in1=rowbufs[m][:, 0:w],
                    )
                    # chunk m complete: reduce its row-candidate buffer
                    nc.vector.reduce_max(
                        out=rowmax[:, m:m + 1],
                        in_=rowbufs[m][:],
                        axis=mybir.AxisListType.X,
                        op=mybir.AluOpType.max,
                    )
                else:
                    blk = blk_pool.tile([P, 2 * NBLK], f16, name="blk",
                                        tag="blk")
                    for h in range(2):
                        ps = psum_pool.tile([P, NBLK], f32, name="ps",
                                            tag="ps")
                        s_h = start + h * NBLK
                        for g in range(KCH // KSTEP):
                            k = g * KSTEP
                            nc.tensor.matmul(
                                ps[:],
                                R[:, k:k + KSTEP, m * P:(m + 1) * P],
                                R[:, k:k + KSTEP, s_h:s_h + NBLK],
                                start=(g == 0), stop=(k + KSTEP == KCH),
                                perf_mode=perf_mode,
                            )
                        nc.scalar.copy(
                            blk[:, h * NBLK:(h + 1) * NBLK], ps[:])
                        if h == 0 and q == 0:
                            # self-similarity at blk[p, p]: -2*scale^2*I
                            nc.vector.tensor_add(
                                out=blk[:, 0:P], in0=blk[:, 0:P],
                                in1=eye[:])
                    # row-candidate accumulate: fold the pair, then merge
                    if q == 0:
                        nc.vector.tensor_max(
                            out=rowbufs[m][:],
                            in0=blk[:, 0:NBLK],
                            in1=blk[:, NBLK:2 * NBLK],
                        )
                    else:
                        nc.vector.tensor_max(
                            out=tmp[:],
                            in0=blk[:, 0:NBLK],
                            in1=blk[:, NBLK:2 * NBLK],
                        )
                        nc.vector.tensor_max(
                            out=rowbufs[m][:],
                            in0=tmp[:],
                            in1=rowbufs[m][:],
                        )
                    # one 1024-wide column-max accumulate
                    nc.vector.tensor_max(
                        out=acc[:, start:start + 2 * NBLK],
                        in0=blk[:],
                        in1=acc[:, start:start + 2 * NBLK],
                    )
                # Everything left of the next item's start is final —
                # stream it out while compute continues.  Flush finely
                # near the end so the last flush is tiny.
                next_start = CW if idx == len(items) - 1 else items[idx + 1][0]
                thresh = 2048 if start < 8192 else 1024
                if next_start - dma_ptr >= thresh or idx == len(items) - 1:
                    # alternate HWDGE queues so end-of-run flushes drain
                    # in parallel instead of serializing on sync
                    dma_eng[flush_i % 2].dma_start(
                        colacc_out[:, dma_ptr:next_start],
                        acc[:, dma_ptr:next_start])
                    flush_i += 1
                    dma_ptr = next_start

            nc.sync.dma_start(rowmax_out[:], rowmax[:])

    nc.compile()
    return nc


def prepare_inputs(x, n_cores=N_CORES):
    """Host prep: normalize (f64), transpose, scale+cast fp8, rotate."""
    xd = np.asarray(x, dtype=np.float64)
    norms = np.sqrt(np.einsum("ij,ij->i", xd, xd))
    np.maximum(norms, 1e-12, out=norms)
    xn = xd / norms[:, None]
    xnt = np.ascontiguousarray(xn.T * FP8_SCALE).astype(ml_dtypes.float8_e4m3)
    negeye = np.ascontiguousarray(
        (-2.0 * FP8_SCALE * FP8_SCALE) * np.eye(P, dtype=np.float16))
    in_maps = []
    for c in range(n_cores):
        s = c * BL
        rot = np.concatenate([xnt[:, s:], xnt[:, :s]], axis=1)[:, :CW]
        in_maps.append({"xt": np.ascontiguousarray(rot), "negeye": negeye})
    return in_maps


def postprocess(results, n_cores=N_CORES):
    """Stitch per-core row/col maxima and apply the scalar epilogue."""
    inv = 1.0 / (FP8_SCALE * FP8_SCALE)
    gmax = np.full(B, -np.inf, dtype=np.float64)
    for c in range(n_cores):
        s = c * BL
        rm = np.asarray(results[c]["rowmax"], dtype=np.float64)   # [P, MCH]
        # local row i = m*P + p  ->  global row s + i
        np.maximum(gmax[s:s + BL], rm.T.reshape(-1), out=gmax[s:s + BL])
        ca = np.asarray(results[c]["colacc"], dtype=np.float64)   # [P, CW]
        colmax = ca.max(axis=0)                                   # [CW]
        # rotated col j -> global row (s + j) mod B; CW < B so no dups
        n0 = min(CW, B - s)
        np.maximum(gmax[s:s + n0], colmax[:n0], out=gmax[s:s + n0])
        if n0 < CW:
            np.maximum(gmax[:CW - n0], colmax[n0:], out=gmax[:CW - n0])
    maxsim = gmax * inv
    d2 = 2.0 - 2.0 * maxsim + EPS
    loss = -0.5 * np.mean(np.log(d2))
    return np.array(loss, dtype=np.float32)


_NC_CACHE = {}


def _get_nc():
    key = (B, D, N_CORES)
    if key not in _NC_CACHE:
        _NC_CACHE[key] = build_nc()
    return _NC_CACHE[key]


def kernel(x, **_ignored):
    nc = _get_nc()
    in_maps = prepare_inputs(x)
    last_exc = None
    for _attempt in range(3):
        try:
            res = run_bass_kernel_spmd(nc, in_maps,
                                       core_ids=list(range(N_CORES)))
            return postprocess(res.results)
        except Exception as exc:  # transient NRT/tunnel hiccups
            last_exc = exc
    raise last_exc


if __name__ == "__main__":
    x = np.random.default_rng(0).standard_normal((B, D), dtype=np.float32)
    print(kernel(x))


# revision 20
# speedup vs baseline: 1.0060x; 1.0060x over previous
"""KoLeo loss kernel for 8 Trainium2 NeuronCores — symmetric-half version.

Reference computation (B=16384, D=1024):
    xn  = x / max(||x||_2, 1e-12)          # row L2-normalize
    sim = xn @ xn.T                        # B x B cosine similarity
    max_sim[i] = max_{j != i} sim[i, j]    # nearest neighbor (excl. self)
    out = -mean(log(sqrt(2 - 2*max_sim + 1e-8)))

Sharding + symmetry: rows of x are split across 8 cores (2048 rows each).
sim is symmetric, so each computed entry sim[i, j] can serve both row i's
max (row-max over the streamed block) and row j's max (column-max,
accumulated across row chunks).  Each 128-row chunk m of a core computes
only an 8320-column window starting at its own diagonal (rotated frame),
instead of the full 16384 columns.  8320 is the provable minimum for a
diagonal-anchored window: a pair {a, b} is missed by chunk(a) iff
(a%128 + delta) mod B in [C, B) and by chunk(b) iff delta in
(b%128, b%128 + B - C]; both fail only if 2C <= B + a%128 + b%128
<= 16638, so C = 8320 (2C = 16640) covers every pair.  This cuts the
matmul work to 0.508x of the full matrix (the true lower bound is 0.5).

Engine pipeline per psum block (width 512; a 128-wide tail per chunk):
    PE   : 4 fp8-DoubleRow matmuls  -> ps [128, w] f32 (PSUM) (853 ns)
    Act  : copy ps -> blk [128, w] fp16 (SBUF)                (~675 ns)
    DVE  : (t==0: blk[:, :128] += -2*scale^2*I)
           rowbuf_m = max(rowbuf_m, blk)   (row-candidate accumulate)
           acc[:, s:s+w] = max(blk, acc)   (column-max)
Both DVE ops are elementwise tensor_max (TensorTensor 2x tier on fp16
SBUF, ~325 ns) — reduce-class DVE ops are capped at 1x (~575 ns), so
the row direction also accumulates elementwise into a per-chunk
[128, 512] buffer and is reduced ONCE per chunk at the end.
The fp16 bounce copy gives the DVE 2-byte SBUF operands (2x/4x tiers;
GPSIMD cannot run TensorTensor ops and cannot read PSUM on TRN2).
Row-max reduces only the fp16 block (never the cross-chunk
accumulator), so it is exact: the accumulator holds other rows' maxima
and must not leak into row-max.

Host finishes: per-column partition max of acc, scatter-max into the
global per-row max (the rotation makes that two slice maxes per core),
then the scalar log/sqrt/mean epilogue in float64.

Implementation notes:
  - Host pre-normalizes rows (f64) and pre-transposes to x.T in fp8e4m3
    scaled by 8 (DoubleRow perf mode, 2x matmul throughput; similarities
    come out scaled by 64, undone on the host).
  - Per-core input is x.T rotated so the core's own rows are columns
    0..2047; chunk m's window starts at column 128*m, so the
    self-similarity diagonal sits at window offset [0, 128) for every
    chunk — one -2*scale^2*I add per chunk kills the self-match.
  - Blocks are processed sorted by window-start column so the resident
    rhs DMA streams strictly left-to-right and finalized slices of the
    column-max accumulator DMA out while compute continues (no tail).
  - One resident SBUF tensor R [128, 8, 10240] (fp8) serves as both
    matmul weights (own rows = cols 0..2047) and moving data.  The
    input DMA streams in ~1-2K-column pieces: wide per-partition
    descriptor runs keep the DMA rings efficient (512-B descriptor
    runs measured 2.4x slower aggregate and stalled the PE ~58 us).
"""

import sys

if "/opt/trn_rl_repo" not in sys.path:
    sys.path.insert(0, "/opt/trn_rl_repo")

import os as _os

import numpy as np
import ml_dtypes

import concourse.bass as bass  # noqa: F401  (import keeps bass registered)
import concourse.mybir as mybir
import concourse.tile as tile
from concourse import bacc
from concourse.bass_utils import run_bass_kernel_spmd

P = 128          # SBUF partitions
NBLK = 512       # similarity column block width (= one PSUM bank of f32)
EPS = 1e-8

B = 16384        # rows of x
D = 1024         # feature dim
N_CORES = 8
BL = B // N_CORES          # local rows per core (2048)
MCH = BL // P              # row chunks per core (16)
WCOLS = 16 * NBLK + P      # window columns per chunk (8320, the minimum)
NT = 17                    # blocks per chunk window (16 full + 128 tail)
CW = (MCH - 1) * P + WCOLS  # resident rotated columns (10240)
KCH = D // P               # contraction chunks (8)
KSTEP = 2                  # fp8 DoubleRow: K chunks of 256

FP8_SCALE = 8.0
NEG_INIT = -60000.0        # fp16-representable, far below any -128..128 sim

# row-reduce flavor: "ts" = tensor_scalar w/ max-accum (fast path),
# "reduce" = plain reduce_max (1x, known-good fallback)
ROWRED = _os.environ.get("KOLEO_ROWRED", "ts")
MEMSET_ENG = _os.environ.get("KOLEO_MEMSET", "vector")  # "gpsimd" | "vector"

# input DMA piece boundaries (columns): fine first, then wide
_PIECES = [0, 1024, 3072, 5120, 7168, CW]


def build_nc(n_cores=N_CORES):
    """Build the per-core SPMD Bass program.

    Inputs :  xt     [D, CW] fp8e4m3 — rotated, normalized, scaled x.T
              negeye [P, P]  f16 — the constant -2*scale^2 * I
    Outputs:  rowmax [P, MCH] f32 — rowmax[p, m] = scale^2 *
              max_{j in window} sim[128m+p, j] (excl. self)
              colacc [P, CW] f16 — colacc[p, c] = scale^2 *
              max over chunks m (with c in window m) of sim[128m+p, c]
    """
    in_dt = mybir.dt.float8e4
    f32 = mybir.dt.float32
    f16 = mybir.dt.float16
    perf_mode = mybir.MatmulPerfMode.DoubleRow

    nc = bacc.Bacc("TRN2", target_bir_lowering=False, debug=False,
                   num_devices=n_cores)
    xt = nc.dram_tensor("xt", [D, CW], in_dt, kind="ExternalInput")
    negeye = nc.dram_tensor("negeye", [P, P], f16, kind="ExternalInput")
    rowmax_out = nc.dram_tensor("rowmax", [P, MCH], f32,
                                kind="ExternalOutput")
    colacc_out = nc.dram_tensor("colacc", [P, CW], f16,
                                kind="ExternalOutput")
    xt_ap = xt[:]

    with tile.TileContext(nc) as tc:
        with (
            tc.tile_pool(name="data", bufs=1) as data_pool,
            tc.tile_pool(name="blk", bufs=8) as blk_pool,
            tc.tile_pool(name="psum", bufs=8, space="PSUM") as psum_pool,
            tc.tile_pool(name="stats", bufs=1) as stats_pool,
        ):
            R = data_pool.tile([P, KCH, CW], in_dt, name="R")
            acc = data_pool.tile([P, CW], f16, name="acc")
            eye = stats_pool.tile([P, P], f16, name="eye")
            rowbufs = [
                stats_pool.tile([P, NBLK], f16, name=f"rowbuf{m}",
                                tag=f"rowbuf{m}")
                for m in range(MCH)
            ]
            rowmax = stats_pool.tile([P, MCH], f32, name="rowmax")

            # col-max accumulator starts far below any similarity
            memset_eng = nc.gpsimd if MEMSET_ENG == "gpsimd" else nc.vector
            memset_eng.memset(acc[:], NEG_INIT)
            nc.sync.dma_start(eye[:], negeye[:])

            # Stream the rotated slab left-to-right on the two HWDGE
            # queues (sync + scalar); gpsimd software DGE stays free.
            dma_eng = [nc.sync, nc.scalar]
            di = 0
            for j in range(len(_PIECES) - 1):
                c0, c1 = _PIECES[j], _PIECES[j + 1]
                for k in range(KCH):
                    dma_eng[di % 2].dma_start(
                        R[:, k, c0:c1], xt_ap[k * P:(k + 1) * P, c0:c1])
                    di += 1

            # Work items sorted by window-start column: each full item is
            # a PAIR of adjacent 512-col blocks of the same chunk (the DVE
            # then folds the pair once, accumulates rowbuf once, and runs
            # ONE 1024-wide column-max — 3 ops/1024 cols instead of 4,
            # cutting per-op overhead), plus one 128-col tail per chunk.
            # The tail's column-max is provably redundant: its pairs
            # (window cols >= 8192 = 8064 + 128 > 8064 + i') are always
            # computed by the partner chunk too, whose row-max covers the
            # column's row (verified exact in numpy).
            items = sorted(
                [(P * m + 1024 * q, m, q, False)
                 for m in range(MCH) for q in range(8)] +
                [(P * m + 8192, m, 8, True) for m in range(MCH)]
            )
            tmp = stats_pool.tile([P, NBLK], f16, name="tmp")
            dma_ptr = 0
            flush_i = 0
            for idx, (start, m, q, is_tail) in enumerate(items):
                lhsT0 = R[:, 0:KSTEP, m * P:(m + 1) * P]
                if is_tail:
                    w = P
                    ps = psum_pool.tile([P, NBLK], f32, name="ps", tag="ps")
                    for g in range(KCH // KSTEP):
                        k = g * KSTEP
                        nc.tensor.matmul(
                            ps[:, 0:w],
                            R[:, k:k + KSTEP, m * P:(m + 1) * P],
                            R[:, k:k + KSTEP, start:start + w],
                            start=(g == 0), stop=(k + KSTEP == KCH),
                            perf_mode=perf_mode,
                        )
                    blk = blk_pool.tile([P, 2 * NBLK], f16, name="blk",
                                        tag="blk")
                    nc.scalar.copy(blk[:, 0:w], ps[:, 0:w])
                    nc.vector.tensor_max(
                        out=rowbufs[m][:, 0:w],
                        in0=blk[:, 0:w],
                        in1=rowbufs[m][:, 0:w],
                    )
                    # chunk m complete: reduce its row-candidate buffer
                    nc.vector.reduce_max(
                        out=rowmax[:, m:m + 1],
                        in_=rowbufs[m][:],
                        axis=mybir.AxisListType.X,
                        op=mybir.AluOpType.max,
                    )
                else:
                    blk = blk_pool.tile([P, 2 * NBLK], f16, name="blk",
                                        tag="blk")
                    for h in range(2):
                        ps = psum_pool.tile([P, NBLK], f32, name="ps",
                                            tag="ps")
                        s_h = start + h * NBLK
                        for g in range(KCH // KSTEP):
                            k = g * KSTEP
                            nc.tensor.matmul(
                                ps[:],
                                R[:, k:k + KSTEP, m * P:(m + 1) * P],
                                R[:, k:k + KSTEP, s_h:s_h + NBLK],
                                start=(g == 0), stop=(k + KSTEP == KCH),
                                perf_mode=perf_mode,
                            )
                        nc.scalar.copy(
                            blk[:, h * NBLK:(h + 1) * NBLK], ps[:])
                        if h == 0 and q == 0:
                            # self-similarity at blk[p, p]: -2*scale^2*I
                            nc.vector.tensor_add(
                                out=blk[:, 0:P], in0=blk[:, 0:P],
                                in1=eye[:])
                    # row-candidate accumulate: fold the pair, then merge
                    if q == 0:
                        nc.vector.tensor_max(
                            out=rowbufs[m][:],
                            in0=blk[:, 0:NBLK],
                            in1=blk[:, NBLK:2 * NBLK],
                        )
                    else:
                        nc.vector.tensor_max(
                            out=tmp[:],
                            in0=blk[:, 0:NBLK],
                            in1=blk[:, NBLK:2 * NBLK],
                        )
                        nc.vector.tensor_max(
                            out=rowbufs[m][:],
                            in0=tmp[:],
                            in1=rowbufs[m][:],
                        )
                    # one 1024-wide column-max accumulate
                    nc.vector.tensor_max(
                        out=acc[:, start:start + 2 * NBLK],
                        in0=blk[:],
                        in1=acc[:, start:start + 2 * NBLK],
                    )
                # Everything left of the next item's start is final —
                # stream it out while compute continues.  Flush finely
                # near the end so the last flush is tiny.
                next_start = CW if idx == len(items) - 1 else items[idx + 1][0]
                thresh = 2048
                if next_start - dma_ptr >= thresh or idx == len(items) - 1:
                    # alternate HWDGE queues so end-of-run flushes drain
                    # in parallel instead of serializing on sync
                    dma_eng[flush_i % 2].dma_start(
                        colacc_out[:, dma_ptr:next_start],
                        acc[:, dma_ptr:next_start])
                    flush_i += 1
                    dma_ptr = next_start

            nc.sync.dma_start(rowmax_out[:], rowmax[:])

    nc.compile()
    return nc


def prepare_inputs(x, n_cores=N_CORES):
    """Host prep: normalize (f64), transpose, scale+cast fp8, rotate."""
    xd = np.asarray(x, dtype=np.float64)
    norms = np.sqrt(np.einsum("ij,ij->i", xd, xd))
    np.maximum(norms, 1e-12, out=norms)
    xn = xd / norms[:, None]
    xnt = np.ascontiguousarray(xn.T * FP8_SCALE).astype(ml_dtypes.float8_e4m3)
    negeye = np.ascontiguousarray(
        (-2.0 * FP8_SCALE * FP8_SCALE) * np.eye(P, dtype=np.float16))
    in_maps = []
    for c in range(n_cores):
        s = c * BL
        rot = np.concatenate([xnt[:, s:], xnt[:, :s]], axis=1)[:, :CW]
        in_maps.append({"xt": np.ascontiguousarray(rot), "negeye": negeye})
    return in_maps


def postprocess(results, n_cores=N_CORES):
    """Stitch per-core row/col maxima and apply the scalar epilogue."""
    inv = 1.0 / (FP8_SCALE * FP8_SCALE)
    gmax = np.full(B, -np.inf, dtype=np.float64)
    for c in range(n_cores):
        s = c * BL
        rm = np.asarray(results[c]["rowmax"], dtype=np.float64)   # [P, MCH]
        # local row i = m*P + p  ->  global row s + i
        np.maximum(gmax[s:s + BL], rm.T.reshape(-1), out=gmax[s:s + BL])
        ca = np.asarray(results[c]["colacc"], dtype=np.float64)   # [P, CW]
        colmax = ca.max(axis=0)                                   # [CW]
        # rotated col j -> global row (s + j) mod B; CW < B so no dups
        n0 = min(CW, B - s)
        np.maximum(gmax[s:s + n0], colmax[:n0], out=gmax[s:s + n0])
        if n0 < CW:
            np.maximum(gmax[:CW - n0], colmax[n0:], out=gmax[:CW - n0])
    maxsim = gmax * inv
    d2 = 2.0 - 2.0 * maxsim + EPS
    loss = -0.5 * np.mean(np.log(d2))
    return np.array(loss, dtype=np.float32)


_NC_CACHE = {}


def _get_nc():
    key = (B, D, N_CORES)
    if key not in _NC_CACHE:
        _NC_CACHE[key] = build_nc()
    return _NC_CACHE[key]


def kernel(x, **_ignored):
    nc = _get_nc()
    in_maps = prepare_inputs(x)
    last_exc = None
    for _attempt in range(3):
        try:
            res = run_bass_kernel_spmd(nc, in_maps,
                                       core_ids=list(range(N_CORES)))
            return postprocess(res.results)
        except Exception as exc:  # transient NRT/tunnel hiccups
            last_exc = exc
    raise last_exc


if __name__ == "__main__":
    x = np.random.default_rng(0).standard_normal((B, D), dtype=np.float32)
    print(kernel(x))
